# revision 10
# baseline (speedup 1.0000x reference)
"""MixIT loss kernel for Trainium2 (8 NeuronCores, Bass/Tile).

Math: reference computes, for each of 16 assignment combinations k,
    mix[k,b,c,t] = sum_s A[k,c,s] * x[b,s,t]        (A tiny [16,2,4])
    loss[k] = sum_b [ snr(mix[k,b,0], m1[b]) + snr(mix[k,b,1], m2[b]) ]
    snr(y, m) = 10*log10(sum_t (y-m)^2 + 30*sum_t y^2) - 10*log10(sum_t y^2)
and returns (argmin_k, min_k).

Since mix is linear in x, every sum over T is a quadratic form in the 6x6
Gram matrix of the per-batch streams {x_0..x_3, m1, m2} over T=64000.  So
the device only computes pairwise dot products; the 16-combination
argmin/min (O(16*32) flops) is finished on host.

Device layout per core (4 batches = 24 streams: 16 x, 4 m1, 4 m2):
T is split as 128 partitions x 500 cols.  Per T-quarter: DMA lands
stream-major tiles zA[128, 24, 125] (contiguous 500B runs per partition),
the idle DVE re-layouts to f-major zB[128, 125, 24], and the PE runs 25
accumulating matmuls with lhsT = rhs = zB[:, 5g:5g+5, :] — a contiguous
[128, 120] operand (24 streams x 5 T-chunks), amortizing the ~35ns fixed
LDWEIGHTS cost that dominates at narrow widths (the BIR verifier requires
the stationary operand to be 2D, hence the f-major re-layout).
out[120,120] accumulates in PSUM; entries with mismatched T-chunk are
junk, and the host sums the 5 aligned diagonal [24,24] blocks:
G[j,k] = sum_f out[24f+j, 24f+k].
"""

import itertools
import sys

import numpy as np

if "/opt/trn_rl_repo" not in sys.path:
    sys.path.insert(0, "/opt/trn_rl_repo")

N_CORES = 8
B = 32               # full batch
S = 4                # estimated sources
T = 64000
BL = B // N_CORES    # batches per core = 4
NJ = 6 * BL          # streams per core = 24 (16 x, 4 m1, 4 m2)
P = 128
COLS = T // P        # 500
FG = 5               # T-chunks fused per matmul (5*24 = 120-col operands)
# Asymmetric T-chunks (in 128-partition columns): a small first chunk lets
# the PE start ~3us earlier; later chunks are sized so DMA stays ahead.
CHUNKS = (100, 125, 125, 150)
NQ = len(CHUNKS)
SNR_MAX = 30.0

_CACHE = {}
LAST_RESULTS = None  # BassKernelResults of the most recent run (for test harness)


def _build_nc():
    from concourse import bacc, bass, tile
    import concourse.mybir as mybir

    nc = bacc.Bacc("TRN2", target_bir_lowering=False, debug=False,
                   num_devices=N_CORES)
    f32 = mybir.dt.float32
    x = nc.dram_tensor("x", [BL, S, T], f32, kind="ExternalInput")
    m1 = nc.dram_tensor("m1", [BL, T], f32, kind="ExternalInput")
    m2 = nc.dram_tensor("m2", [BL, T], f32, kind="ExternalInput")
    g = nc.dram_tensor("g", [NJ * FG, NJ * FG], f32, kind="ExternalOutput")

    with tile.TileContext(nc) as tc:
        with (
            tc.tile_pool(name="za", bufs=4) as zapool,
            tc.tile_pool(name="zb", bufs=4) as zbpool,
            tc.tile_pool(name="ps", bufs=1, space=bass.MemorySpace.PSUM) as psp,
            tc.tile_pool(name="o", bufs=1) as opool,
        ):
            acc = psp.tile([NJ * FG, NJ * FG], f32)
            ncopy = 0
            c0 = 0
            for q, cq in enumerate(CHUNKS):
                za = zapool.tile([P, NJ, max(CHUNKS)], f32, tag="za")
                # Alternate the two HWDGE rings (sync & scalar sequencers) so
                # descriptor generation is not serialized on one engine.
                e1, e2 = (nc.sync, nc.scalar) if q % 2 == 0 else (nc.scalar,
                                                                  nc.sync)
                e1.dma_start(
                    out=za[:, 0:16, 0:cq],
                    in_=x.ap().rearrange("b s (p c) -> p (b s) c",
                                         p=P)[:, :, c0:c0 + cq],
                )
                e2.dma_start(
                    out=za[:, 16:20, 0:cq],
                    in_=m1.ap().rearrange("b (p c) -> p b c",
                                          p=P)[:, :, c0:c0 + cq],
                )
                e2.dma_start(
                    out=za[:, 20:24, 0:cq],
                    in_=m2.ap().rearrange("b (p c) -> p b c",
                                          p=P)[:, :, c0:c0 + cq],
                )
                zb = zbpool.tile([P, max(CHUNKS), NJ], f32, tag="zb")
                # Re-layout in 25-col chunks, alternating DVE/GpSimd, so the
                # PE can start on chunk 0 while later chunks still copy.
                CH = 25
                for k in range(cq // CH):
                    src = za[:, :, CH * k:CH * (k + 1)].transpose([0, 2, 1])
                    dst = zb[:, CH * k:CH * (k + 1), :]
                    eng = nc.vector if ncopy % 2 == 0 else nc.gpsimd
                    ncopy += 1
                    eng.tensor_copy(dst, src)
                for i in range(cq // FG):
                    op = zb[:, FG * i:FG * (i + 1), :]
                    nc.tensor.matmul(
                        acc[:, :], op, op,
                        start=(q == 0 and i == 0),
                        stop=(q == NQ - 1 and i == cq // FG - 1),
                    )
                c0 += cq
            gout = opool.tile([NJ * FG, NJ * FG], f32)
            nc.vector.tensor_copy(gout[:, :], acc[:, :])
            nc.sync.dma_start(out=g.ap()[:, :], in_=gout[:, :])
    nc.compile()
    return nc


def _get_nc():
    if "nc" not in _CACHE:
        _CACHE["nc"] = _build_nc()
    return _CACHE["nc"]


def _finish_host(grams: np.ndarray):
    """grams: [N_CORES, 120, 120] per-core PE blocks -> (argmin, min)."""
    # Collapse the fused T-chunk axis: G[j,k] = sum_f out[24f+j, 24f+k].
    g5 = grams.reshape(N_CORES, FG, NJ, FG, NJ).astype(np.float64)
    g24 = np.einsum("cfjfk->cjk", g5)

    # Per full-batch index b: core c = b // BL, local l = b % BL.
    # Stream layout per core: x_(l,s) at 4*l+s, m1_l at 16+l, m2_l at 20+l.
    Gxx = np.empty((B, S, S), np.float64)   # sum_t x_s x_s'
    C1 = np.empty((B, S), np.float64)       # sum_t x_s m1
    C2 = np.empty((B, S), np.float64)
    M1 = np.empty((B,), np.float64)         # sum_t m1^2
    M2 = np.empty((B,), np.float64)
    for b in range(B):
        c, l = divmod(b, BL)
        gm = g24[c]
        xs = slice(S * l, S * l + S)
        Gxx[b] = gm[xs, xs]
        C1[b] = gm[xs, 16 + l]
        C2[b] = gm[xs, 20 + l]
        M1[b] = gm[16 + l, 16 + l]
        M2[b] = gm[20 + l, 20 + l]

    combos = np.array(list(itertools.product([0, 1], repeat=S)), np.float64)
    losses = np.zeros(len(combos), np.float64)
    with np.errstate(divide="ignore"):
        for w, cc, mm in ((combos, C1, M1), (1.0 - combos, C2, M2)):
            bq = np.einsum("ks,bst,kt->kb", w, Gxx, w)        # sum_t y^2
            aq = bq - 2.0 * (w @ cc.T) + mm[None, :]          # sum_t (y-m)^2
            losses += np.sum(10.0 * np.log10(aq + SNR_MAX * bq)
                             - 10.0 * np.log10(bq), axis=1)
    k = int(np.argmin(losses))
    return np.int32(k), np.float32(losses[k])


def kernel(estimated_sources: np.ndarray, m1: np.ndarray, m2: np.ndarray):
    global LAST_RESULTS
    from concourse.bass_utils import run_bass_kernel_spmd

    x = np.ascontiguousarray(estimated_sources, dtype=np.float32)
    m1 = np.ascontiguousarray(m1, dtype=np.float32)
    m2 = np.ascontiguousarray(m2, dtype=np.float32)

    in_maps = []
    for c in range(N_CORES):
        sl = slice(BL * c, BL * (c + 1))
        in_maps.append({
            "x": np.ascontiguousarray(x[sl]),
            "m1": np.ascontiguousarray(m1[sl]),
            "m2": np.ascontiguousarray(m2[sl]),
        })

    nc = _get_nc()
    LAST_RESULTS = run_bass_kernel_spmd(nc, in_maps, list(range(N_CORES)))
    grams = np.stack([LAST_RESULTS.results[c]["g"] for c in range(N_CORES)])
    return _finish_host(grams)
